# revision 5
# baseline (speedup 1.0000x reference)
"""Trainium2 Bass kernel for the CAM-drop attention module.

Computes, per sample n:
    cams  = relu(w @ x)            # [Cout=4, HW]   (1x1 conv over Cin=4096)
    thr   = gama * max_hw(cams)    # [4, 1]
    drop  = where(cams > thr, 0, cams)
    mean  = sum_o(drop) / 4        # [1, HW]
    out   = x * mean               # [Cin, HW]

Sharding: data-parallel over batch N=32 across 8 NeuronCores (4 samples each).

v2 design (vs the fp32/stream baseline at ~235us):
  - The drop threshold compare is numerically fragile (min |cams-thr|/thr is
    ~2.4e-5 in this input set), so the conv MUST consume fp32 x. But the
    final product is tolerant: writing out (and doing the final multiply) in
    fp16 adds only ~6e-4 max rel err vs the 2e-2 gate.
  - Loads: x fp32 (51.4 MB/core, irreducible). Stores: fp16 (25.7 MB/core,
    half of baseline). Total HBM traffic 77 MB/core vs 103 MB.
  - Conv as "xstat": camsT = x.T @ wT with the x chunk stationary on the PE
    and the tiny [128,4] w tile moving -> PE time ~weight-load-bound,
    ~3x less than streaming x through at fp32's 4 cyc/row.
  - ACT converts each x tile to fp16 as it lands (idle engine); DVE does the
    final multiply in fp16 at its 2x 16-bit rate, in place on the fp16 copy;
    ACT-ring DMAs store the fp16 tiles.
"""

import numpy as np
from contextlib import ExitStack

import concourse.bass as bass
import concourse.bacc as bacc
import concourse.tile as tile
from concourse import mybir
from concourse.bass_utils import run_bass_kernel_spmd
from concourse.masks import make_identity
from concourse.tile_rust import add_dep_helper

# Problem geometry (hardcoded per the grading contract).
N_TOTAL, CIN, H, W = 32, 4096, 28, 28
HW = H * W            # 784
COUT = 4
N_CORES = 8
N_PER_CORE = N_TOTAL // N_CORES   # 4
P = 128
NCHUNKS = CIN // P    # 32 partition-chunks of Cin
QCH = 4               # chunks per DMA transfer (1.6 MB)
NT = NCHUNKS // QCH   # 8 tiles per sample
NSPLIT = 512          # PSUM-bank split of the HW free dim: 512 + 272
F32 = mybir.dt.float32
F16 = mybir.dt.float16
# hw blocks for the xstat stationary tiles: six 128-wide + one 16-wide so
# transpose outputs never cross a PSUM bank boundary.
XSTAT_BLKS = [(i * P, P) for i in range(6)] + [(6 * P, HW - 6 * P)]


def build_cam_body(ctx: ExitStack, tc: "tile.TileContext", out_ap, x_ap, w_ap,
                   g_ap, iters=1):
    """Emit the kernel body. x_ap: [N_PER_CORE, CIN, HW] f32 DRAM,
    out_ap: [N_PER_CORE, CIN, HW] f16 DRAM, w_ap: [COUT, CIN] f32 DRAM,
    g_ap: [1, 1] f32 DRAM."""
    nc = tc.nc

    xpool = ctx.enter_context(tc.tile_pool(name="xq", bufs=6))
    x16pool = ctx.enter_context(tc.tile_pool(name="x16", bufs=2 * NT - 1))
    small = ctx.enter_context(tc.tile_pool(name="small", bufs=1))
    cpool = ctx.enter_context(tc.tile_pool(name="cams", bufs=2))
    mpool = ctx.enter_context(tc.tile_pool(name="mean", bufs=2))
    ps_ct = ctx.enter_context(tc.tile_pool(name="ps_camsT", bufs=2, space="PSUM"))
    ps_c = ctx.enter_context(tc.tile_pool(name="ps_cams", bufs=2, space="PSUM"))
    ps_b = ctx.enter_context(tc.tile_pool(name="ps_bcast", bufs=1, space="PSUM"))

    # ---- one-time setup: transpose w to [Cin, Cout] layout, constants ----
    # w lives in DRAM as [4, 4096]; the matmul needs per-chunk moving tiles of
    # shape [128 (Cin slice), 4]. A direct DMA of that layout would be
    # element-granular, so load [4, 4096] and transpose on the PE.
    wsb = xpool.tile([COUT, CIN], F32, tag="xq")
    nc.sync.dma_start(out=wsb, in_=w_ap)

    ident = small.tile([P, P], F32)
    make_identity(nc, ident)

    wt_ps = ps_b.tile([P, NCHUNKS * COUT], F32, tag="bps")
    for k in range(NCHUNKS):
        nc.tensor.transpose(
            wt_ps[:, k * COUT:(k + 1) * COUT],
            wsb[:, k * P:(k + 1) * P],
            ident[0:COUT, 0:COUT],
        )
    wt = small.tile([P, NCHUNKS, COUT], F32)
    nc.vector.tensor_copy(wt, wt_ps.rearrange("p (k o) -> p k o", o=COUT))

    # 0.25 * ones[4, 128]: the channel-sum + partition-broadcast matmul weight.
    ones = small.tile([COUT, P], F32)
    nc.vector.memset(ones, 0.25)

    # gama broadcast to partitions 0..3.
    gsb = small.tile([COUT, 1], F32)
    nc.gpsimd.dma_start(out=gsb, in_=g_ap.to_broadcast([COUT, 1]))

    # ---- per-sample pipeline ----
    for s in [s for _ in range(iters) for s in range(N_PER_CORE)]:
        xs = x_ap[s].rearrange("(k p) hw -> p k hw", p=P)    # [128, 32, 784]
        os_ = out_ap[s].rearrange("(k p) hw -> p k hw", p=P)

        # camsT[hw, o] = x[:, hw].T @ wT: x blocks stationary, w moving.
        # All 7 block-groups live in ONE PSUM bank. start=True marks the
        # whole 2KB bank pending-zero, so only the bank's FIRST matmul may
        # carry start=True; every other block's k==0 write then lands on
        # pending-zero bytes and initializes (overwrites) its own region.
        # Explicit scheduler deps pin the bank-start matmul first.
        camsT_ps = ps_ct.tile([P, 8 * COUT], F32)
        # initialize the corners the 7 block-groups never write (cols 28-31
        # and the tail block's partitions 16-127) so the copy below reads
        # fully-initialized memory.
        nc.vector.memset(camsT_ps, 0.0)
        bank_start = None
        x16s = []
        for t in range(NT):
            xt = xpool.tile([P, QCH, HW], F32, tag="xq")
            nc.sync.dma_start(out=xt, in_=xs[:, t * QCH:(t + 1) * QCH, :])
            x16 = x16pool.tile([P, QCH, HW], F16, tag="x16")
            nc.scalar.copy(x16, xt)
            x16s.append(x16)
            for j in range(QCH):
                k = t * QCH + j
                rhs = wt[:, k, :]
                xk = xt[:, j, :]
                for b, (off, blk) in enumerate(XSTAT_BLKS):
                    first = k == 0 and b == 0
                    last = k == NCHUNKS - 1 and b == len(XSTAT_BLKS) - 1
                    mm = nc.tensor.matmul(
                        camsT_ps[0:blk, b * COUT:(b + 1) * COUT],
                        xk[:, off:off + blk], rhs, start=first, stop=last,
                        skip_group_check=True)
                    if first:
                        bank_start = mm.ins
                    elif k == 0:
                        add_dep_helper(mm.ins, bank_start, sync=False,
                                       reason="psum bank pending-zero start order")
        camsT_sb = cpool.tile([P, 8 * COUT], F32)
        nc.vector.tensor_copy(camsT_sb, camsT_ps)
        # transpose camsT back to cams[4, HW] on the PE; same single-start
        # rule per destination bank (blocks 0-3 -> bank 0, 4-6 -> bank 1).
        cams_ps = ps_c.tile([COUT, 1024], F32)  # 4 KB -> two PSUM banks
        tp_start = {}
        for b, (off, blk) in enumerate(XSTAT_BLKS):
            bank = off // NSPLIT
            tp = nc.tensor.matmul(
                cams_ps[:, off:off + blk],
                camsT_sb[0:blk, b * COUT:(b + 1) * COUT],
                ident[0:blk, 0:blk],
                is_transpose=True,
                start=bank not in tp_start,
                stop=(b == 3 or b == len(XSTAT_BLKS) - 1),
                skip_group_check=True)
            if bank not in tp_start:
                tp_start[bank] = tp.ins
            else:
                add_dep_helper(tp.ins, tp_start[bank], sync=False,
                               reason="psum bank pending-zero start order")

        # relu -> spatial max -> threshold -> drop -> channel sum (+broadcast).
        # ACT computes relu(cams) while DVE reduces the raw max concurrently;
        # thr = gama * max(raw_max, 0) == gama * max(relu(cams)).
        cams_sb = cpool.tile([COUT, HW], F32)
        nc.scalar.activation(cams_sb, cams_ps[:, 0:HW],
                             mybir.ActivationFunctionType.Relu)
        cmax = cpool.tile([COUT, 1], F32)
        nc.vector.tensor_reduce(cmax, cams_ps[:, 0:HW],
                                axis=mybir.AxisListType.X,
                                op=mybir.AluOpType.max)
        thr = cpool.tile([COUT, 1], F32)
        nc.vector.tensor_scalar(thr, cmax, 0.0, gsb,
                                op0=mybir.AluOpType.max,
                                op1=mybir.AluOpType.mult)
        dropped = cpool.tile([COUT, HW], F32)
        # dropped = (cams <= thr) * cams
        nc.vector.scalar_tensor_tensor(dropped, cams_sb, thr, cams_sb,
                                       op0=mybir.AluOpType.is_le,
                                       op1=mybir.AluOpType.mult)

        # bps[p, hw] = 0.25 * sum_o dropped[o, hw], replicated to 128 rows.
        bps = ps_b.tile([P, 1024], F32, tag="bps")
        nc.tensor.matmul(bps[:, 0:NSPLIT], ones, dropped[:, 0:NSPLIT],
                         start=True, stop=True)
        nc.tensor.matmul(bps[:, NSPLIT:HW], ones, dropped[:, NSPLIT:HW],
                         start=True, stop=True)
        mean16 = mpool.tile([P, HW], F16, tag="m16")
        nc.vector.tensor_copy(mean16, bps[:, 0:HW])

        # out = x16 * mean16 in place on the fp16 tiles (2x DVE rate), then
        # store on the ACT ring so stores don't queue behind next-sample
        # loads on the SP ring.
        mb = mean16.unsqueeze(1).to_broadcast([P, QCH, HW])
        for t in range(NT):
            nc.vector.tensor_mul(x16s[t], x16s[t], mb)
            nc.scalar.dma_start(out=os_[:, t * QCH:(t + 1) * QCH, :],
                                in_=x16s[t])


def build_module(iters=1):
    """iters > 1 unrolls the whole body multiple times inside one NEFF —
    used only by the timing harness to amortize dispatch overhead."""
    nc = bacc.Bacc(trn_type="TRN2", num_devices=N_CORES, name="cam_drop")
    x = nc.dram_tensor("x", [N_PER_CORE, CIN, HW], F32, kind="ExternalInput").ap()
    w = nc.dram_tensor("w", [COUT, CIN], F32, kind="ExternalInput").ap()
    g = nc.dram_tensor("gama", [1, 1], F32, kind="ExternalInput").ap()
    out = nc.dram_tensor("out", [N_PER_CORE, CIN, HW], F16,
                         kind="ExternalOutput").ap()
    with tile.TileContext(nc) as tc:
        with ExitStack() as ctx:
            build_cam_body(ctx, tc, out, x, w, g, iters=iters)
    nc.compile()
    return nc


_cached_module = None


def make_in_maps(x, fc_weights, gama):
    """Host-side prep: shard FULL inputs into per-core input maps."""
    xs = np.ascontiguousarray(
        np.asarray(x, dtype=np.float32).reshape(N_TOTAL, CIN, HW))
    w = np.ascontiguousarray(
        np.asarray(fc_weights, dtype=np.float32).reshape(COUT, CIN))
    g = np.asarray(gama, dtype=np.float32).reshape(1, 1)
    return [
        {"x": np.ascontiguousarray(xs[i * N_PER_CORE:(i + 1) * N_PER_CORE]),
         "w": w, "gama": g}
        for i in range(N_CORES)
    ]


def assemble_out(outs):
    """Host-side post: full (N_TOTAL, CIN, HW) fp32 from gathered outputs."""
    return np.asarray(outs["out"], dtype=np.float32).reshape(N_TOTAL, CIN, HW)


def run(x, fc_weights, gama, trace=False):
    """Shard inputs over 8 cores, run, gather. Returns (output, BassKernelResults)."""
    global _cached_module
    if _cached_module is None:
        _cached_module = build_module()
    nc = _cached_module

    in_maps = make_in_maps(x, fc_weights, gama)
    if trace:
        try:  # this container's antenv has no axon NTFF hook
            from antenv.axon_hooks import get_axon_ntff_profile_hook  # noqa: F401
        except ImportError:
            trace = False
    res = run_bass_kernel_spmd(nc, in_maps, core_ids=list(range(N_CORES)),
                               trace=trace)
    full = assemble_out(
        {"out": np.concatenate([r["out"] for r in res.results], axis=0)})
    return full.reshape(N_TOTAL, CIN, H, W), res


def kernel(x, fc_weights, gama):
    out, _ = run(x, fc_weights, gama, trace=False)
    return out
